# revision 16
# baseline (speedup 1.0000x reference)
"""Original baseline kernel (reconstructed) — device-wedge canary."""
import sys
sys.path.insert(0, "/opt/trn_rl_repo")

import math
import numpy as np
import ml_dtypes

import concourse.bass as bass
import concourse.tile as tile
from concourse import bacc, mybir
from concourse.masks import make_identity

bf16 = ml_dtypes.bfloat16
F32 = mybir.dt.float32
BF = mybir.dt.bfloat16
AF = mybir.ActivationFunctionType
ALU = mybir.AluOpType

D = 2048
NH = 16
DH = 128
NCORES = 8
HPC = NH // NCORES
DLOC = HPC * DH
EPS = 1e-6
TB = 512
SQRT_DH = math.sqrt(DH)

_BUILD_CACHE = {}


def _build(T):
    NTB = T // TB
    nc = bacc.Bacc("TRN2", target_bir_lowering=False)

    xt_in = nc.dram_tensor("xt", [D, T], BF, kind="ExternalInput")
    wq_in = nc.dram_tensor("wqkv", [D, 3 * DLOC], BF, kind="ExternalInput")
    wp_in = nc.dram_tensor("wproj", [DLOC, D], BF, kind="ExternalInput")
    ve_in = nc.dram_tensor("ve", [T, DLOC], BF, kind="ExternalInput")
    cos_in = nc.dram_tensor("cos", [T, 32], F32, kind="ExternalInput")
    sin_in = nc.dram_tensor("sin", [T, 32], F32, kind="ExternalInput")
    mask_in = nc.dram_tensor("mask", [128, 4, TB], BF, kind="ExternalInput")
    out_d = nc.dram_tensor("out", [T, D], F32, kind="ExternalOutput")

    with tile.TileContext(nc) as tc:
        with (
            tc.tile_pool(name="const", bufs=1) as const,
            tc.tile_pool(name="res", bufs=1) as res,
            tc.tile_pool(name="xt", bufs=2) as xtp,
            tc.tile_pool(name="work", bufs=2) as work,
            tc.tile_pool(name="att", bufs=3) as att,
            tc.tile_pool(name="accp", bufs=2) as accp,
            tc.tile_pool(name="prj", bufs=2) as prj,
            tc.tile_pool(name="psA", bufs=2, space="PSUM") as psA,
            tc.tile_pool(name="psB", bufs=1, space="PSUM") as psB,
            tc.tile_pool(name="psC", bufs=1, space="PSUM") as psC,
            tc.tile_pool(name="psD", bufs=2, space="PSUM") as psD,
        ):
            wq_sb = const.tile([128, D // 128, 3 * DLOC], BF, tag="wq")
            nc.sync.dma_start(wq_sb[:], wq_in.rearrange("(c p) e -> p c e", p=128))
            wp_sb = const.tile([128, HPC, D], BF, tag="wp")
            nc.sync.dma_start(wp_sb[:], wp_in.rearrange("(h p) e -> p h e", p=128))
            cos_sb = const.tile([128, T // 128, 32], F32, tag="cos")
            nc.sync.dma_start(cos_sb[:], cos_in.rearrange("(c p) f -> p c f", p=128))
            sin_sb = const.tile([128, T // 128, 32], F32, tag="sin")
            nc.sync.dma_start(sin_sb[:], sin_in.rearrange("(c p) f -> p c f", p=128))
            mask_sb = const.tile([128, 4, TB], BF, tag="mask")
            nc.sync.dma_start(mask_sb[:], mask_in[:])
            ident = const.tile([128, 128], BF, tag="ident")
            make_identity(nc, ident[:])
            ones = const.tile([128, 1], F32, tag="ones")
            nc.vector.memset(ones[:], 1.0)
            lnbias = const.tile([128, 1], F32, tag="lnbias")
            nc.vector.memset(lnbias[:], float(EPS * SQRT_DH))

            qT = [res.tile([128, HPC, TB], BF, tag=f"qT{i}", name=f"qT{i}") for i in range(NTB)]
            kT = [res.tile([128, HPC, TB], BF, tag=f"kT{i}", name=f"kT{i}") for i in range(NTB)]
            vB = [res.tile([128, 4, DLOC], BF, tag=f"v{i}", name=f"v{i}") for i in range(NTB)]
            for ti in range(NTB):
                t0 = ti * TB
                xt = xtp.tile([128, D // 128, TB], BF, tag="xt")
                nc.sync.dma_start(
                    xt[:], xt_in[:, t0:t0 + TB].rearrange("(c p) t -> p c t", p=128))

                for sub in range(4):
                    tg = ti * 4 + sub
                    qkv_ps = psA.tile([128, 1024], F32, tag="big")
                    ndc = D // 128
                    for dc in range(ndc):
                        lhsT = xt[:, dc, sub * 128:(sub + 1) * 128]
                        st, sp = dc == 0, dc == ndc - 1
                        nc.tensor.matmul(qkv_ps[:, 0:512], lhsT, wq_sb[:, dc, 0:512], start=st, stop=sp)
                        nc.tensor.matmul(qkv_ps[:, 512:768], lhsT, wq_sb[:, dc, 512:768], start=st, stop=sp)
                    nc.any.tensor_copy(vB[ti][:, sub, :], qkv_ps[:, 512:768])
                    ssq = work.tile([128, 4], F32, tag="ssq")
                    sq_scr = work.tile([128, 512], BF, tag="sqscr")
                    for i in range(4):
                        nc.scalar.activation(
                            sq_scr[:, i * 128:(i + 1) * 128], qkv_ps[:, i * 128:(i + 1) * 128],
                            AF.Square, accum_out=ssq[:, i:i + 1])
                    rstd = work.tile([128, 4], F32, tag="rstd")
                    nc.scalar.activation(rstd[:], ssq[:], AF.Ln,
                                         scale=float(SQRT_DH / DH), bias=lnbias[:])
                    nc.scalar.activation(rstd[:], rstd[:], AF.Exp, scale=-0.5)
                    qn = work.tile([128, HPC, DH], BF, tag="qn")
                    kn = work.tile([128, HPC, DH], BF, tag="kn")
                    for h in range(HPC):
                        nc.any.tensor_scalar_mul(qn[:, h, :], qkv_ps[:, h * 128:(h + 1) * 128], rstd[:, h:h + 1])
                        nc.any.tensor_scalar_mul(kn[:, h, :], qkv_ps[:, 256 + h * 128:256 + (h + 1) * 128], rstd[:, 2 + h:3 + h])
                    cosb = cos_sb[:, tg, :][:, None, :].broadcast_to([128, HPC, 32])
                    sinb = sin_sb[:, tg, :][:, None, :].broadcast_to([128, HPC, 32])
                    for tl in (qn, kn):
                        x1 = tl[:, :, 0:32]
                        x2 = tl[:, :, 64:96]
                        r1 = work.tile([128, HPC, 32], BF, tag="r1")
                        r2 = work.tile([128, HPC, 32], BF, tag="r2")
                        r3 = work.tile([128, HPC, 32], BF, tag="r3")
                        r4 = work.tile([128, HPC, 32], BF, tag="r4")
                        nc.vector.tensor_mul(r1[:], x1, cosb)
                        nc.vector.tensor_mul(r2[:], x2, sinb)
                        nc.vector.tensor_mul(r3[:], x1, sinb)
                        nc.vector.tensor_mul(r4[:], x2, cosb)
                        nc.vector.tensor_add(x1, r1[:], r2[:])
                        nc.vector.tensor_sub(x2, r4[:], r3[:])
                    for h in range(HPC):
                        for tl, dstl in ((qn, qT), (kn, kT)):
                            tp = psC.tile([128, 128], BF, tag="tp")
                            nc.tensor.transpose(tp[:], tl[:, h, :], ident[:])
                            nc.any.tensor_copy(dstl[ti][:, h, sub * 128:(sub + 1) * 128], tp[:])
                nc.gpsimd.dma_start(
                    vB[ti][:], ve_in[t0:t0 + TB, :].rearrange("(c p) d -> p c d", p=128),
                    accum_op=ALU.add)

                linv = prj.tile([128, HPC, 4], F32, tag="linv")
                oB = prj.tile([128, HPC, TB], BF, tag="o")
                ns = (ti + 1) * 4
                for h in range(HPC):
                    def emit_scores(i):
                        sc = psA.tile([128, 1024], F32, tag="big")
                        for k2 in range(2):
                            sj = 2 * i + k2
                            blk, sb_ = sj // 4, sj % 4
                            nc.tensor.matmul(
                                sc[:, k2 * 512:(k2 + 1) * 512],
                                kT[blk][:, h, sb_ * 128:(sb_ + 1) * 128],
                                qT[ti][:, h, :], start=True, stop=True)
                        return sc

                    l_acc = accp.tile([128, TB], F32, tag="lacc")
                    nc.vector.memset(l_acc[:], 0.0)
                    o_ps = psB.tile([128, TB], F32, tag="o")
                    niter = ns // 2
                    sc_cur = emit_scores(0)
                    for i in range(niter):
                        sc_next = emit_scores(i + 1) if i + 1 < niter else None
                        probs = att.tile([128, 1024], BF, tag="probs")
                        nc.scalar.activation(probs[:], sc_cur[:], AF.Exp)
                        for k2 in range(2):
                            j = 2 * i + k2 - ti * 4
                            if j >= 0:
                                nc.vector.tensor_mul(
                                    probs[:, k2 * 512:(k2 + 1) * 512],
                                    probs[:, k2 * 512:(k2 + 1) * 512], mask_sb[:, j, :])
                        nc.vector.tensor_add(l_acc[:], l_acc[:], probs[:, 0:512])
                        nc.vector.tensor_add(l_acc[:], l_acc[:], probs[:, 512:1024])
                        for k2 in range(2):
                            sj = 2 * i + k2
                            blk, sb_ = sj // 4, sj % 4
                            nc.tensor.matmul(
                                o_ps[:], vB[blk][:, sb_, h * 128:(h + 1) * 128],
                                probs[:, k2 * 512:(k2 + 1) * 512],
                                start=(sj == 0), stop=(sj == ns - 1))
                        sc_cur = sc_next
                    lcol = psC.tile([128, 4], F32, tag="tp")
                    for c in range(4):
                        nc.tensor.matmul(lcol[:, c:c + 1], l_acc[:, c * 128:(c + 1) * 128],
                                         ones[:], start=(c == 0), stop=(c == 3))
                    nc.vector.reciprocal(linv[:, h, :], lcol[:])
                    nc.any.tensor_copy(oB[:, h, :], o_ps[:])

                for sub in range(4):
                    out_sb = prj.tile([128, D], F32, tag="outsb")
                    for dn in range(D // 512):
                        pr0 = psD.tile([128, 512], F32, tag="pr")
                        nc.tensor.matmul(pr0[:], oB[:, 0, sub * 128:(sub + 1) * 128],
                                         wp_sb[:, 0, dn * 512:(dn + 1) * 512], start=True, stop=True)
                        tmp = prj.tile([128, 512], F32, tag="tmp")
                        nc.any.tensor_scalar_mul(tmp[:], pr0[:], linv[:, 0, sub:sub + 1])
                        pr1 = psD.tile([128, 512], F32, tag="pr")
                        nc.tensor.matmul(pr1[:], oB[:, 1, sub * 128:(sub + 1) * 128],
                                         wp_sb[:, 1, dn * 512:(dn + 1) * 512], start=True, stop=True)
                        nc.vector.scalar_tensor_tensor(
                            out_sb[:, dn * 512:(dn + 1) * 512], pr1[:], linv[:, 1, sub:sub + 1],
                            tmp[:], op0=ALU.mult, op1=ALU.add)
                    nc.sync.dma_start(out_d[t0 + sub * 128: t0 + (sub + 1) * 128, :], out_sb[:])
    return nc


def _host_prep(x, ve, lambdas, qkv_w, proj_w, T):
    x = np.asarray(x, np.float32).reshape(T, D)
    xt = np.ascontiguousarray(x.T).astype(bf16)  # [D, T] pre-transposed
    ve = np.asarray(ve, np.float32).reshape(T, NH * DH)
    lam = np.asarray(lambdas, np.float32)
    qkv_w = np.asarray(qkv_w, np.float32)
    proj_w = np.asarray(proj_w, np.float32)

    quarter = DH // 4
    ang = (1.0 / 1024.0) ** np.linspace(0.0, 1.0, quarter, dtype=np.float32)
    theta = np.arange(T, dtype=np.float32)[:, None] * ang[None, :]
    cos_t = np.cos(theta).astype(np.float32)
    sin_t = np.sin(theta).astype(np.float32)

    s_l = np.arange(128)[:, None]
    t_l = np.arange(TB)[None, :]
    mask = np.stack([(t_l >= s_l + 128 * j) for j in range(4)], axis=1).astype(bf16)

    in_maps = []
    for c in range(NCORES):
        sl = slice(c * DLOC, (c + 1) * DLOC)
        wqkv = np.concatenate(
            [qkv_w[0, sl].T, qkv_w[1, sl].T, lam[0] * qkv_w[2, sl].T], axis=1)
        in_maps.append({
            "xt": xt,
            "wqkv": np.ascontiguousarray(wqkv).astype(bf16),
            "wproj": np.ascontiguousarray(proj_w[:, sl].T).astype(bf16),
            "ve": np.ascontiguousarray(lam[1] * ve[:, sl]).astype(bf16),
            "cos": cos_t, "sin": sin_t, "mask": mask,
        })
    return in_maps


def kernel(x, ve, lambdas, qkv_w, proj_w):
    B, T, _ = x.shape
    in_maps = _host_prep(x, ve, lambdas, qkv_w, proj_w, T)
    if T not in _BUILD_CACHE:
        nc = _build(T)
        nc.compile()
        _BUILD_CACHE[T] = nc
    nc = _BUILD_CACHE[T]

    from concourse.bass_utils import run_bass_kernel_spmd
    res = run_bass_kernel_spmd(nc, in_maps, core_ids=list(range(NCORES)))
    out = np.zeros((T, D), np.float32)
    for c in range(NCORES):
        out += res.results[c]["out"]
    return out.reshape(B, T, D)


# revision 21
# speedup vs baseline: 1.0084x; 1.0084x over previous
"""Original baseline kernel (reconstructed) — device-wedge canary."""
import sys
sys.path.insert(0, "/opt/trn_rl_repo")

import math
import numpy as np
import ml_dtypes

import concourse.bass as bass
import concourse.tile as tile
from concourse import bacc, mybir
from concourse.masks import make_identity

bf16 = ml_dtypes.bfloat16
F32 = mybir.dt.float32
BF = mybir.dt.bfloat16
AF = mybir.ActivationFunctionType
ALU = mybir.AluOpType

D = 2048
NH = 16
DH = 128
NCORES = 8
HPC = NH // NCORES
DLOC = HPC * DH
EPS = 1e-6
TB = 512
SQRT_DH = math.sqrt(DH)

_BUILD_CACHE = {}


def _build(T):
    NTB = T // TB
    nc = bacc.Bacc("TRN2", target_bir_lowering=False)

    xt_in = nc.dram_tensor("xt", [D, T], BF, kind="ExternalInput")
    wq_in = nc.dram_tensor("wqkv", [D, 3 * DLOC], BF, kind="ExternalInput")
    wp_in = nc.dram_tensor("wproj", [DLOC, D], BF, kind="ExternalInput")
    ve_in = nc.dram_tensor("ve", [T, DLOC], BF, kind="ExternalInput")
    cos_in = nc.dram_tensor("cos", [T, 32], F32, kind="ExternalInput")
    sin_in = nc.dram_tensor("sin", [T, 32], F32, kind="ExternalInput")
    mask_in = nc.dram_tensor("mask", [128, 4, TB], BF, kind="ExternalInput")
    out_d = nc.dram_tensor("out", [T, D], F32, kind="ExternalOutput")

    with tile.TileContext(nc) as tc:
        with (
            tc.tile_pool(name="const", bufs=1) as const,
            tc.tile_pool(name="res", bufs=1) as res,
            tc.tile_pool(name="xt", bufs=2) as xtp,
            tc.tile_pool(name="work", bufs=2) as work,
            tc.tile_pool(name="att", bufs=3) as att,
            tc.tile_pool(name="accp", bufs=2) as accp,
            tc.tile_pool(name="prj", bufs=2) as prj,
            tc.tile_pool(name="psA", bufs=2, space="PSUM") as psA,
            tc.tile_pool(name="psB", bufs=1, space="PSUM") as psB,
            tc.tile_pool(name="psC", bufs=1, space="PSUM") as psC,
            tc.tile_pool(name="psD", bufs=2, space="PSUM") as psD,
        ):
            wq_sb = const.tile([128, D // 128, 3 * DLOC], BF, tag="wq")
            nc.sync.dma_start(wq_sb[:], wq_in.rearrange("(c p) e -> p c e", p=128))
            wp_sb = const.tile([128, HPC, D], BF, tag="wp")
            nc.sync.dma_start(wp_sb[:], wp_in.rearrange("(h p) e -> p h e", p=128))
            cos_sb = const.tile([128, T // 128, 32], F32, tag="cos")
            nc.sync.dma_start(cos_sb[:], cos_in.rearrange("(c p) f -> p c f", p=128))
            sin_sb = const.tile([128, T // 128, 32], F32, tag="sin")
            nc.sync.dma_start(sin_sb[:], sin_in.rearrange("(c p) f -> p c f", p=128))
            mask_sb = const.tile([128, 4, TB], BF, tag="mask")
            nc.sync.dma_start(mask_sb[:], mask_in[:])
            ident = const.tile([128, 128], BF, tag="ident")
            make_identity(nc, ident[:])
            ones = const.tile([128, 1], F32, tag="ones")
            nc.vector.memset(ones[:], 1.0)
            lnbias = const.tile([128, 1], F32, tag="lnbias")
            nc.vector.memset(lnbias[:], float(EPS * SQRT_DH))

            qT = [res.tile([128, HPC, TB], BF, tag=f"qT{i}", name=f"qT{i}") for i in range(NTB)]
            kT = [res.tile([128, HPC, TB], BF, tag=f"kT{i}", name=f"kT{i}") for i in range(NTB)]
            vB = [res.tile([128, 4, DLOC], BF, tag=f"v{i}", name=f"v{i}") for i in range(NTB)]
            for ti in range(NTB):
                t0 = ti * TB
                xt = xtp.tile([128, D // 128, TB], BF, tag="xt")
                nc.sync.dma_start(
                    xt[:], xt_in[:, t0:t0 + TB].rearrange("(c p) t -> p c t", p=128))

                def emit_transposes(qn, kn, sub):
                    # batched: 4 PE transposes into one psum tile, 2 copies out
                    tp = psC.tile([128, 4, 128], BF, tag="tp")
                    for h in range(HPC):
                        nc.tensor.transpose(tp[:, h, :], qn[:, h, :], ident[:])
                        nc.tensor.transpose(tp[:, 2 + h, :], kn[:, h, :], ident[:])
                    nc.any.tensor_copy(qT[ti][:, :, sub * 128:(sub + 1) * 128], tp[:, 0:2, :])
                    nc.any.tensor_copy(kT[ti][:, :, sub * 128:(sub + 1) * 128], tp[:, 2:4, :])

                pend = None  # previous sub's (qn, kn, sub): transpose after next MMs
                for sub in range(4):
                    tg = ti * 4 + sub
                    qkv_ps = psA.tile([128, 1024], F32, tag="big")
                    ndc = D // 128
                    for dc in range(ndc):
                        lhsT = xt[:, dc, sub * 128:(sub + 1) * 128]
                        st, sp = dc == 0, dc == ndc - 1
                        nc.tensor.matmul(qkv_ps[:, 0:512], lhsT, wq_sb[:, dc, 0:512], start=st, stop=sp)
                        nc.tensor.matmul(qkv_ps[:, 512:768], lhsT, wq_sb[:, dc, 512:768], start=st, stop=sp)
                    if pend is not None:
                        emit_transposes(*pend)
                    nc.any.tensor_copy(vB[ti][:, sub, :], qkv_ps[:, 512:768])
                    ssq = work.tile([128, 4], F32, tag="ssq")
                    sq_scr = work.tile([128, 512], BF, tag="sqscr")
                    for i in range(4):
                        nc.scalar.activation(
                            sq_scr[:, i * 128:(i + 1) * 128], qkv_ps[:, i * 128:(i + 1) * 128],
                            AF.Square, accum_out=ssq[:, i:i + 1])
                    rstd = work.tile([128, 4], F32, tag="rstd")
                    nc.scalar.activation(rstd[:], ssq[:], AF.Ln,
                                         scale=float(SQRT_DH / DH), bias=lnbias[:])
                    nc.scalar.activation(rstd[:], rstd[:], AF.Exp, scale=-0.5)
                    qn = work.tile([128, HPC, DH], BF, tag="qn")
                    kn = work.tile([128, HPC, DH], BF, tag="kn")
                    for h in range(HPC):
                        nc.any.tensor_scalar_mul(qn[:, h, :], qkv_ps[:, h * 128:(h + 1) * 128], rstd[:, h:h + 1])
                        nc.any.tensor_scalar_mul(kn[:, h, :], qkv_ps[:, 256 + h * 128:256 + (h + 1) * 128], rstd[:, 2 + h:3 + h])
                    cosb = cos_sb[:, tg, :][:, None, :].broadcast_to([128, HPC, 32])
                    sinb = sin_sb[:, tg, :][:, None, :].broadcast_to([128, HPC, 32])
                    for tl in (qn, kn):
                        x1 = tl[:, :, 0:32]
                        x2 = tl[:, :, 64:96]
                        r1 = work.tile([128, HPC, 32], BF, tag="r1")
                        r2 = work.tile([128, HPC, 32], BF, tag="r2")
                        r3 = work.tile([128, HPC, 32], BF, tag="r3")
                        r4 = work.tile([128, HPC, 32], BF, tag="r4")
                        nc.vector.tensor_mul(r1[:], x1, cosb)
                        nc.vector.tensor_mul(r2[:], x2, sinb)
                        nc.vector.tensor_mul(r3[:], x1, sinb)
                        nc.vector.tensor_mul(r4[:], x2, cosb)
                        nc.vector.tensor_add(x1, r1[:], r2[:])
                        nc.vector.tensor_sub(x2, r4[:], r3[:])
                    pend = (qn, kn, sub)
                emit_transposes(*pend)
                nc.gpsimd.dma_start(
                    vB[ti][:], ve_in[t0:t0 + TB, :].rearrange("(c p) d -> p c d", p=128),
                    accum_op=ALU.add)

                linv = prj.tile([128, HPC, 4], F32, tag="linv")
                oB = prj.tile([128, HPC, TB], BF, tag="o")
                ns = (ti + 1) * 4
                for h in range(HPC):
                    def emit_scores(i):
                        sc = psA.tile([128, 1024], F32, tag="big")
                        for k2 in range(2):
                            sj = 2 * i + k2
                            blk, sb_ = sj // 4, sj % 4
                            nc.tensor.matmul(
                                sc[:, k2 * 512:(k2 + 1) * 512],
                                kT[blk][:, h, sb_ * 128:(sb_ + 1) * 128],
                                qT[ti][:, h, :], start=True, stop=True)
                        return sc

                    l_acc = accp.tile([128, TB], F32, tag="lacc")
                    o_ps = psB.tile([128, TB], F32, tag="o")
                    niter = ns // 2
                    sc_cur = emit_scores(0)
                    for i in range(niter):
                        sc_next = emit_scores(i + 1) if i + 1 < niter else None
                        probs = att.tile([128, 1024], BF, tag="probs")
                        nc.scalar.activation(probs[:], sc_cur[:], AF.Exp)
                        for k2 in range(2):
                            j = 2 * i + k2 - ti * 4
                            if j >= 0:
                                nc.vector.tensor_mul(
                                    probs[:, k2 * 512:(k2 + 1) * 512],
                                    probs[:, k2 * 512:(k2 + 1) * 512], mask_sb[:, j, :])
                        if i == 0:
                            nc.vector.tensor_add(l_acc[:], probs[:, 0:512], probs[:, 512:1024])
                        else:
                            nc.vector.tensor_add(l_acc[:], l_acc[:], probs[:, 0:512])
                            nc.vector.tensor_add(l_acc[:], l_acc[:], probs[:, 512:1024])
                        for k2 in range(2):
                            sj = 2 * i + k2
                            blk, sb_ = sj // 4, sj % 4
                            nc.tensor.matmul(
                                o_ps[:], vB[blk][:, sb_, h * 128:(h + 1) * 128],
                                probs[:, k2 * 512:(k2 + 1) * 512],
                                start=(sj == 0), stop=(sj == ns - 1))
                        sc_cur = sc_next
                    lcol = psC.tile([128, 4], F32, tag="tp")
                    for c in range(4):
                        nc.tensor.matmul(lcol[:, c:c + 1], l_acc[:, c * 128:(c + 1) * 128],
                                         ones[:], start=(c == 0), stop=(c == 3))
                    nc.vector.reciprocal(linv[:, h, :], lcol[:])
                    nc.any.tensor_copy(oB[:, h, :], o_ps[:])

                for sub in range(4):
                    out_sb = prj.tile([128, D], F32, tag="outsb")
                    for dn in range(D // 512):
                        pr0 = psD.tile([128, 512], F32, tag="pr")
                        nc.tensor.matmul(pr0[:], oB[:, 0, sub * 128:(sub + 1) * 128],
                                         wp_sb[:, 0, dn * 512:(dn + 1) * 512], start=True, stop=True)
                        tmp = prj.tile([128, 512], F32, tag="tmp")
                        nc.any.tensor_scalar_mul(tmp[:], pr0[:], linv[:, 0, sub:sub + 1])
                        pr1 = psD.tile([128, 512], F32, tag="pr")
                        nc.tensor.matmul(pr1[:], oB[:, 1, sub * 128:(sub + 1) * 128],
                                         wp_sb[:, 1, dn * 512:(dn + 1) * 512], start=True, stop=True)
                        nc.vector.scalar_tensor_tensor(
                            out_sb[:, dn * 512:(dn + 1) * 512], pr1[:], linv[:, 1, sub:sub + 1],
                            tmp[:], op0=ALU.mult, op1=ALU.add)
                    nc.sync.dma_start(out_d[t0 + sub * 128: t0 + (sub + 1) * 128, :], out_sb[:])
    return nc


def _host_prep(x, ve, lambdas, qkv_w, proj_w, T):
    x = np.asarray(x, np.float32).reshape(T, D)
    xt = np.ascontiguousarray(x.T).astype(bf16)  # [D, T] pre-transposed
    ve = np.asarray(ve, np.float32).reshape(T, NH * DH)
    lam = np.asarray(lambdas, np.float32)
    qkv_w = np.asarray(qkv_w, np.float32)
    proj_w = np.asarray(proj_w, np.float32)

    quarter = DH // 4
    ang = (1.0 / 1024.0) ** np.linspace(0.0, 1.0, quarter, dtype=np.float32)
    theta = np.arange(T, dtype=np.float32)[:, None] * ang[None, :]
    cos_t = np.cos(theta).astype(np.float32)
    sin_t = np.sin(theta).astype(np.float32)

    s_l = np.arange(128)[:, None]
    t_l = np.arange(TB)[None, :]
    mask = np.stack([(t_l >= s_l + 128 * j) for j in range(4)], axis=1).astype(bf16)

    in_maps = []
    for c in range(NCORES):
        sl = slice(c * DLOC, (c + 1) * DLOC)
        wqkv = np.concatenate(
            [qkv_w[0, sl].T, qkv_w[1, sl].T, lam[0] * qkv_w[2, sl].T], axis=1)
        in_maps.append({
            "xt": xt,
            "wqkv": np.ascontiguousarray(wqkv).astype(bf16),
            "wproj": np.ascontiguousarray(proj_w[:, sl].T).astype(bf16),
            "ve": np.ascontiguousarray(lam[1] * ve[:, sl]).astype(bf16),
            "cos": cos_t, "sin": sin_t, "mask": mask,
        })
    return in_maps


def kernel(x, ve, lambdas, qkv_w, proj_w):
    B, T, _ = x.shape
    in_maps = _host_prep(x, ve, lambdas, qkv_w, proj_w, T)
    if T not in _BUILD_CACHE:
        nc = _build(T)
        nc.compile()
        _BUILD_CACHE[T] = nc
    nc = _BUILD_CACHE[T]

    from concourse.bass_utils import run_bass_kernel_spmd
    res = run_bass_kernel_spmd(nc, in_maps, core_ids=list(range(NCORES)))
    out = np.zeros((T, D), np.float32)
    for c in range(NCORES):
        out += res.results[c]["out"]
    return out.reshape(B, T, D)


# revision 23
# speedup vs baseline: 1.1433x; 1.1337x over previous
"""Original baseline kernel (reconstructed) — device-wedge canary."""
import sys
sys.path.insert(0, "/opt/trn_rl_repo")

import math
import numpy as np
import ml_dtypes

import concourse.bass as bass
import concourse.tile as tile
from concourse import bacc, mybir
from concourse.masks import make_identity

bf16 = ml_dtypes.bfloat16
F32 = mybir.dt.float32
BF = mybir.dt.bfloat16
AF = mybir.ActivationFunctionType
ALU = mybir.AluOpType

D = 2048
NH = 16
DH = 128
NCORES = 8
HPC = NH // NCORES
DLOC = HPC * DH
EPS = 1e-6
TB = 512
SQRT_DH = math.sqrt(DH)

_BUILD_CACHE = {}


def _build(T):
    NTB = T // TB
    nc = bacc.Bacc("TRN2", target_bir_lowering=False)

    xt_in = nc.dram_tensor("xt", [D, T], BF, kind="ExternalInput")
    wq_in = nc.dram_tensor("wqkv", [D, 3 * DLOC], BF, kind="ExternalInput")
    wp_in = nc.dram_tensor("wproj", [DLOC, D], BF, kind="ExternalInput")
    ve_in = nc.dram_tensor("ve", [T, DLOC], BF, kind="ExternalInput")
    cos_in = nc.dram_tensor("cos", [T, 32], F32, kind="ExternalInput")
    sin_in = nc.dram_tensor("sin", [T, 32], F32, kind="ExternalInput")
    mask_in = nc.dram_tensor("mask", [128, 4, TB], BF, kind="ExternalInput")
    out_d = nc.dram_tensor("out", [T, D], F32, kind="ExternalOutput")

    with tile.TileContext(nc) as tc:
        with (
            tc.tile_pool(name="const", bufs=1) as const,
            tc.tile_pool(name="res", bufs=1) as res,
            tc.tile_pool(name="xt", bufs=2) as xtp,
            tc.tile_pool(name="work", bufs=2) as work,
            tc.tile_pool(name="att", bufs=3) as att,
            tc.tile_pool(name="accp", bufs=2) as accp,
            tc.tile_pool(name="prj", bufs=2) as prj,
            tc.tile_pool(name="psA", bufs=2, space="PSUM") as psA,
            tc.tile_pool(name="psB", bufs=1, space="PSUM") as psB,
            tc.tile_pool(name="psC", bufs=1, space="PSUM") as psC,
            tc.tile_pool(name="psD", bufs=2, space="PSUM") as psD,
        ):
            wq_sb = const.tile([128, D // 128, 3 * DLOC], BF, tag="wq")
            nc.sync.dma_start(wq_sb[:], wq_in.rearrange("(c p) e -> p c e", p=128))
            wp_sb = const.tile([128, HPC, D], BF, tag="wp")
            nc.sync.dma_start(wp_sb[:], wp_in.rearrange("(h p) e -> p h e", p=128))
            cos_sb = const.tile([128, T // 128, 32], F32, tag="cos")
            nc.sync.dma_start(cos_sb[:], cos_in.rearrange("(c p) f -> p c f", p=128))
            sin_sb = const.tile([128, T // 128, 32], F32, tag="sin")
            nc.sync.dma_start(sin_sb[:], sin_in.rearrange("(c p) f -> p c f", p=128))
            mask_sb = const.tile([128, 4, TB], BF, tag="mask")
            nc.sync.dma_start(mask_sb[:], mask_in[:])
            ident = const.tile([128, 128], BF, tag="ident")
            make_identity(nc, ident[:])
            ones = const.tile([128, 1], F32, tag="ones")
            nc.vector.memset(ones[:], 1.0)
            lnbias = const.tile([128, 1], F32, tag="lnbias")
            nc.vector.memset(lnbias[:], float(EPS * SQRT_DH))

            qT = [res.tile([128, HPC, TB], BF, tag=f"qT{i}", name=f"qT{i}") for i in range(NTB)]
            kT = [res.tile([128, HPC, TB], BF, tag=f"kT{i}", name=f"kT{i}") for i in range(NTB)]
            vB = [res.tile([128, 4, DLOC], BF, tag=f"v{i}", name=f"v{i}") for i in range(NTB)]
            for ti in range(NTB):
                t0 = ti * TB
                xt = xtp.tile([128, D // 128, TB], BF, tag="xt")
                nc.sync.dma_start(
                    xt[:], xt_in[:, t0:t0 + TB].rearrange("(c p) t -> p c t", p=128))

                def emit_transposes(qn, kn, sub):
                    # batched: 4 PE transposes into one psum tile, 2 copies out
                    tp = psC.tile([128, 4, 128], BF, tag="tp")
                    for h in range(HPC):
                        nc.tensor.transpose(tp[:, h, :], qn[:, h, :], ident[:])
                        nc.tensor.transpose(tp[:, 2 + h, :], kn[:, h, :], ident[:])
                    nc.any.tensor_copy(qT[ti][:, :, sub * 128:(sub + 1) * 128], tp[:, 0:2, :])
                    nc.any.tensor_copy(kT[ti][:, :, sub * 128:(sub + 1) * 128], tp[:, 2:4, :])

                pend = None  # previous sub's (qn, kn, sub): transpose after next MMs
                for sub in range(4):
                    tg = ti * 4 + sub
                    qkv_ps = psA.tile([128, 1024], F32, tag="big")
                    ndc = D // 128
                    for dc in range(ndc):
                        lhsT = xt[:, dc, sub * 128:(sub + 1) * 128]
                        st, sp = dc == 0, dc == ndc - 1
                        nc.tensor.matmul(qkv_ps[:, 0:512], lhsT, wq_sb[:, dc, 0:512], start=st, stop=sp)
                        nc.tensor.matmul(qkv_ps[:, 512:768], lhsT, wq_sb[:, dc, 512:768], start=st, stop=sp)
                    if pend is not None:
                        emit_transposes(*pend)
                    nc.any.tensor_copy(vB[ti][:, sub, :], qkv_ps[:, 512:768])
                    ssq = work.tile([128, 4], F32, tag="ssq")
                    sq_scr = work.tile([128, 512], BF, tag="sqscr")
                    for i in range(4):
                        nc.scalar.activation(
                            sq_scr[:, i * 128:(i + 1) * 128], qkv_ps[:, i * 128:(i + 1) * 128],
                            AF.Square, accum_out=ssq[:, i:i + 1])
                    rstd_s = work.tile([128, 4], F32, tag="rstds")
                    nc.scalar.activation(rstd_s[:], ssq[:], AF.Sqrt,
                                         scale=float(SQRT_DH / DH), bias=lnbias[:])
                    rstd = work.tile([128, 4], F32, tag="rstd")
                    nc.vector.reciprocal(rstd[:], rstd_s[:])
                    qn = work.tile([128, HPC, DH], BF, tag="qn")
                    kn = work.tile([128, HPC, DH], BF, tag="kn")
                    for h in range(HPC):
                        nc.vector.tensor_scalar_mul(qn[:, h, :], qkv_ps[:, h * 128:(h + 1) * 128], rstd[:, h:h + 1])
                        nc.vector.tensor_scalar_mul(kn[:, h, :], qkv_ps[:, 256 + h * 128:256 + (h + 1) * 128], rstd[:, 2 + h:3 + h])
                    cosb = cos_sb[:, tg, :][:, None, :].broadcast_to([128, HPC, 32])
                    sinb = sin_sb[:, tg, :][:, None, :].broadcast_to([128, HPC, 32])
                    for tl in (qn, kn):
                        x1 = tl[:, :, 0:32]
                        x2 = tl[:, :, 64:96]
                        r1 = work.tile([128, HPC, 32], BF, tag="r1")
                        r2 = work.tile([128, HPC, 32], BF, tag="r2")
                        r3 = work.tile([128, HPC, 32], BF, tag="r3")
                        r4 = work.tile([128, HPC, 32], BF, tag="r4")
                        nc.vector.tensor_mul(r1[:], x1, cosb)
                        nc.vector.tensor_mul(r2[:], x2, sinb)
                        nc.vector.tensor_mul(r3[:], x1, sinb)
                        nc.vector.tensor_mul(r4[:], x2, cosb)
                        nc.vector.tensor_add(x1, r1[:], r2[:])
                        nc.vector.tensor_sub(x2, r4[:], r3[:])
                    pend = (qn, kn, sub)
                emit_transposes(*pend)
                nc.gpsimd.dma_start(
                    vB[ti][:], ve_in[t0:t0 + TB, :].rearrange("(c p) d -> p c d", p=128),
                    accum_op=ALU.add)

                linv = prj.tile([128, HPC, 4], F32, tag="linv")
                oB = prj.tile([128, HPC, TB], BF, tag="o")
                ns = (ti + 1) * 4
                for h in range(HPC):
                    def emit_scores(i):
                        sc = psA.tile([128, 1024], F32, tag="big")
                        for k2 in range(2):
                            sj = 2 * i + k2
                            blk, sb_ = sj // 4, sj % 4
                            nc.tensor.matmul(
                                sc[:, k2 * 512:(k2 + 1) * 512],
                                kT[blk][:, h, sb_ * 128:(sb_ + 1) * 128],
                                qT[ti][:, h, :], start=True, stop=True)
                        return sc

                    l_acc = accp.tile([128, TB], F32, tag="lacc")
                    o_ps = psB.tile([128, TB], F32, tag="o")
                    niter = ns // 2
                    sc_cur = emit_scores(0)
                    for i in range(niter):
                        sc_next = emit_scores(i + 1) if i + 1 < niter else None
                        probs = att.tile([128, 1024], BF, tag="probs")
                        nc.scalar.activation(probs[:], sc_cur[:], AF.Exp)
                        for k2 in range(2):
                            j = 2 * i + k2 - ti * 4
                            if j >= 0:
                                nc.vector.tensor_mul(
                                    probs[:, k2 * 512:(k2 + 1) * 512],
                                    probs[:, k2 * 512:(k2 + 1) * 512], mask_sb[:, j, :])
                        if i == 0:
                            nc.vector.tensor_add(l_acc[:], probs[:, 0:512], probs[:, 512:1024])
                        else:
                            nc.vector.tensor_add(l_acc[:], l_acc[:], probs[:, 0:512])
                            nc.vector.tensor_add(l_acc[:], l_acc[:], probs[:, 512:1024])
                        for k2 in range(2):
                            sj = 2 * i + k2
                            blk, sb_ = sj // 4, sj % 4
                            nc.tensor.matmul(
                                o_ps[:], vB[blk][:, sb_, h * 128:(h + 1) * 128],
                                probs[:, k2 * 512:(k2 + 1) * 512],
                                start=(sj == 0), stop=(sj == ns - 1))
                        sc_cur = sc_next
                    lcol = psC.tile([128, 4], F32, tag="tp")
                    for c in range(4):
                        nc.tensor.matmul(lcol[:, c:c + 1], l_acc[:, c * 128:(c + 1) * 128],
                                         ones[:], start=(c == 0), stop=(c == 3))
                    nc.vector.reciprocal(linv[:, h, :], lcol[:])
                    nc.any.tensor_copy(oB[:, h, :], o_ps[:])

                for sub in range(4):
                    out_sb = prj.tile([128, D], F32, tag="outsb")
                    for dn in range(D // 512):
                        pr0 = psD.tile([128, 512], F32, tag="pr")
                        nc.tensor.matmul(pr0[:], oB[:, 0, sub * 128:(sub + 1) * 128],
                                         wp_sb[:, 0, dn * 512:(dn + 1) * 512], start=True, stop=True)
                        tmp = prj.tile([128, 512], F32, tag="tmp")
                        nc.any.tensor_scalar_mul(tmp[:], pr0[:], linv[:, 0, sub:sub + 1])
                        pr1 = psD.tile([128, 512], F32, tag="pr")
                        nc.tensor.matmul(pr1[:], oB[:, 1, sub * 128:(sub + 1) * 128],
                                         wp_sb[:, 1, dn * 512:(dn + 1) * 512], start=True, stop=True)
                        nc.vector.scalar_tensor_tensor(
                            out_sb[:, dn * 512:(dn + 1) * 512], pr1[:], linv[:, 1, sub:sub + 1],
                            tmp[:], op0=ALU.mult, op1=ALU.add)
                    nc.sync.dma_start(out_d[t0 + sub * 128: t0 + (sub + 1) * 128, :], out_sb[:])
    return nc


def _host_prep(x, ve, lambdas, qkv_w, proj_w, T):
    x = np.asarray(x, np.float32).reshape(T, D)
    xt = np.ascontiguousarray(x.T).astype(bf16)  # [D, T] pre-transposed
    ve = np.asarray(ve, np.float32).reshape(T, NH * DH)
    lam = np.asarray(lambdas, np.float32)
    qkv_w = np.asarray(qkv_w, np.float32)
    proj_w = np.asarray(proj_w, np.float32)

    quarter = DH // 4
    ang = (1.0 / 1024.0) ** np.linspace(0.0, 1.0, quarter, dtype=np.float32)
    theta = np.arange(T, dtype=np.float32)[:, None] * ang[None, :]
    cos_t = np.cos(theta).astype(np.float32)
    sin_t = np.sin(theta).astype(np.float32)

    s_l = np.arange(128)[:, None]
    t_l = np.arange(TB)[None, :]
    mask = np.stack([(t_l >= s_l + 128 * j) for j in range(4)], axis=1).astype(bf16)

    in_maps = []
    for c in range(NCORES):
        sl = slice(c * DLOC, (c + 1) * DLOC)
        wqkv = np.concatenate(
            [qkv_w[0, sl].T, qkv_w[1, sl].T, lam[0] * qkv_w[2, sl].T], axis=1)
        in_maps.append({
            "xt": xt,
            "wqkv": np.ascontiguousarray(wqkv).astype(bf16),
            "wproj": np.ascontiguousarray(proj_w[:, sl].T).astype(bf16),
            "ve": np.ascontiguousarray(lam[1] * ve[:, sl]).astype(bf16),
            "cos": cos_t, "sin": sin_t, "mask": mask,
        })
    return in_maps


def kernel(x, ve, lambdas, qkv_w, proj_w):
    B, T, _ = x.shape
    in_maps = _host_prep(x, ve, lambdas, qkv_w, proj_w, T)
    if T not in _BUILD_CACHE:
        nc = _build(T)
        nc.compile()
        _BUILD_CACHE[T] = nc
    nc = _BUILD_CACHE[T]

    from concourse.bass_utils import run_bass_kernel_spmd
    res = run_bass_kernel_spmd(nc, in_maps, core_ids=list(range(NCORES)))
    out = np.zeros((T, D), np.float32)
    for c in range(NCORES):
        out += res.results[c]["out"]
    return out.reshape(B, T, D)


# revision 25
# speedup vs baseline: 1.1724x; 1.0255x over previous
"""Original baseline kernel (reconstructed) — device-wedge canary."""
import sys
sys.path.insert(0, "/opt/trn_rl_repo")

import math
import numpy as np
import ml_dtypes

import concourse.bass as bass
import concourse.tile as tile
from concourse import bacc, mybir
from concourse.masks import make_identity

bf16 = ml_dtypes.bfloat16
F32 = mybir.dt.float32
BF = mybir.dt.bfloat16
AF = mybir.ActivationFunctionType
ALU = mybir.AluOpType

D = 2048
NH = 16
DH = 128
NCORES = 8
HPC = NH // NCORES
DLOC = HPC * DH
EPS = 1e-6
TB = 512
SQRT_DH = math.sqrt(DH)

_BUILD_CACHE = {}


def _build(T):
    NTB = T // TB
    nc = bacc.Bacc("TRN2", target_bir_lowering=False)

    xt_in = nc.dram_tensor("xt", [D, T], BF, kind="ExternalInput")
    wq_in = nc.dram_tensor("wqkv", [D, 3 * DLOC], BF, kind="ExternalInput")
    wp_in = nc.dram_tensor("wproj", [DLOC, D], BF, kind="ExternalInput")
    ve_in = nc.dram_tensor("ve", [T, DLOC], BF, kind="ExternalInput")
    cos_in = nc.dram_tensor("cos", [T, 32], F32, kind="ExternalInput")
    sin_in = nc.dram_tensor("sin", [T, 32], F32, kind="ExternalInput")
    mask_in = nc.dram_tensor("mask", [128, 4, TB], BF, kind="ExternalInput")
    out_d = nc.dram_tensor("out", [T, D], F32, kind="ExternalOutput")

    with tile.TileContext(nc) as tc:
        with (
            tc.tile_pool(name="const", bufs=1) as const,
            tc.tile_pool(name="res", bufs=1) as res,
            tc.tile_pool(name="xt", bufs=2) as xtp,
            tc.tile_pool(name="work", bufs=2) as work,
            tc.tile_pool(name="att", bufs=3) as att,
            tc.tile_pool(name="accp", bufs=2) as accp,
            tc.tile_pool(name="prj", bufs=2) as prj,
            tc.tile_pool(name="psA", bufs=2, space="PSUM") as psA,
            tc.tile_pool(name="psB", bufs=1, space="PSUM") as psB,
            tc.tile_pool(name="psC", bufs=1, space="PSUM") as psC,
            tc.tile_pool(name="psD", bufs=2, space="PSUM") as psD,
        ):
            wq_sb = const.tile([128, D // 128, 3 * DLOC], BF, tag="wq")
            nc.sync.dma_start(wq_sb[:], wq_in.rearrange("(c p) e -> p c e", p=128))
            wp_sb = const.tile([128, HPC, D], BF, tag="wp")
            nc.sync.dma_start(wp_sb[:], wp_in.rearrange("(h p) e -> p h e", p=128))
            cos_sb = const.tile([128, T // 128, 32], F32, tag="cos")
            nc.sync.dma_start(cos_sb[:], cos_in.rearrange("(c p) f -> p c f", p=128))
            sin_sb = const.tile([128, T // 128, 32], F32, tag="sin")
            nc.sync.dma_start(sin_sb[:], sin_in.rearrange("(c p) f -> p c f", p=128))
            mask_sb = const.tile([128, 4, TB], BF, tag="mask")
            nc.sync.dma_start(mask_sb[:], mask_in[:])
            ident = const.tile([128, 128], BF, tag="ident")
            make_identity(nc, ident[:])
            ones = const.tile([128, 1], F32, tag="ones")
            nc.vector.memset(ones[:], 1.0)
            lnbias = const.tile([128, 1], F32, tag="lnbias")
            nc.vector.memset(lnbias[:], float(EPS * SQRT_DH))

            qT = [res.tile([128, HPC, TB], BF, tag=f"qT{i}", name=f"qT{i}") for i in range(NTB)]
            kT = [res.tile([128, HPC, TB], BF, tag=f"kT{i}", name=f"kT{i}") for i in range(NTB)]
            vB = [res.tile([128, 4, DLOC], BF, tag=f"v{i}", name=f"v{i}") for i in range(NTB)]
            for ti in range(NTB):
                t0 = ti * TB
                xt = xtp.tile([128, D // 128, TB], BF, tag="xt")
                nc.sync.dma_start(
                    xt[:], xt_in[:, t0:t0 + TB].rearrange("(c p) t -> p c t", p=128))

                def emit_transposes(qn, kn, sub):
                    # batched: 4 PE transposes into one psum tile, 2 copies out
                    tp = psC.tile([128, 4, 128], BF, tag="tp")
                    for h in range(HPC):
                        nc.tensor.transpose(tp[:, h, :], qn[:, h, :], ident[:])
                        nc.tensor.transpose(tp[:, 2 + h, :], kn[:, h, :], ident[:])
                    nc.any.tensor_copy(qT[ti][:, :, sub * 128:(sub + 1) * 128], tp[:, 0:2, :])
                    nc.any.tensor_copy(kT[ti][:, :, sub * 128:(sub + 1) * 128], tp[:, 2:4, :])

                pend = None  # previous sub's (qn, kn, sub): transpose after next MMs
                for sub in range(4):
                    tg = ti * 4 + sub
                    qkv_ps = psA.tile([128, 1024], F32, tag="big")
                    ndc = D // 128
                    for dc in range(ndc):
                        lhsT = xt[:, dc, sub * 128:(sub + 1) * 128]
                        st, sp = dc == 0, dc == ndc - 1
                        nc.tensor.matmul(qkv_ps[:, 0:512], lhsT, wq_sb[:, dc, 0:512], start=st, stop=sp)
                        nc.tensor.matmul(qkv_ps[:, 512:768], lhsT, wq_sb[:, dc, 512:768], start=st, stop=sp)
                    if pend is not None:
                        emit_transposes(*pend)
                    nc.any.tensor_copy(vB[ti][:, sub, :], qkv_ps[:, 512:768])
                    # evacuate raw q|k to SBUF; sumsq via one DVE square + reduce
                    qkr = work.tile([128, 4, DH], BF, tag="qkr")
                    nc.scalar.copy(qkr[:], qkv_ps[:, 0:512])
                    sq_scr = work.tile([128, 4, DH], BF, tag="sqscr")
                    nc.vector.tensor_mul(sq_scr[:], qkr[:], qkr[:])
                    ssq = work.tile([128, 4], F32, tag="ssq")
                    nc.vector.tensor_reduce(ssq[:], sq_scr[:],
                                            axis=mybir.AxisListType.X, op=ALU.add)
                    rstd_s = work.tile([128, 4], F32, tag="rstds")
                    nc.scalar.activation(rstd_s[:], ssq[:], AF.Sqrt,
                                         scale=float(SQRT_DH / DH), bias=lnbias[:])
                    rstd = work.tile([128, 4], F32, tag="rstd")
                    nc.vector.reciprocal(rstd[:], rstd_s[:])
                    qn = work.tile([128, HPC, DH], BF, tag="qn")
                    kn = work.tile([128, HPC, DH], BF, tag="kn")
                    for h in range(HPC):
                        nc.vector.tensor_scalar_mul(qn[:, h, :], qkr[:, h, :], rstd[:, h:h + 1])
                        nc.vector.tensor_scalar_mul(kn[:, h, :], qkr[:, 2 + h, :], rstd[:, 2 + h:3 + h])
                    cosb = cos_sb[:, tg, :][:, None, :].broadcast_to([128, HPC, 32])
                    sinb = sin_sb[:, tg, :][:, None, :].broadcast_to([128, HPC, 32])
                    for tl in (qn, kn):
                        x1 = tl[:, :, 0:32]
                        x2 = tl[:, :, 64:96]
                        r1 = work.tile([128, HPC, 32], BF, tag="r1")
                        r2 = work.tile([128, HPC, 32], BF, tag="r2")
                        r3 = work.tile([128, HPC, 32], BF, tag="r3")
                        r4 = work.tile([128, HPC, 32], BF, tag="r4")
                        nc.vector.tensor_mul(r1[:], x1, cosb)
                        nc.vector.tensor_mul(r2[:], x2, sinb)
                        nc.vector.tensor_mul(r3[:], x1, sinb)
                        nc.vector.tensor_mul(r4[:], x2, cosb)
                        nc.vector.tensor_add(x1, r1[:], r2[:])
                        nc.vector.tensor_sub(x2, r4[:], r3[:])
                    pend = (qn, kn, sub)
                emit_transposes(*pend)
                nc.gpsimd.dma_start(
                    vB[ti][:], ve_in[t0:t0 + TB, :].rearrange("(c p) d -> p c d", p=128),
                    accum_op=ALU.add)

                linv = prj.tile([128, HPC, 4], F32, tag="linv")
                oB = prj.tile([128, HPC, TB], BF, tag="o")
                ns = (ti + 1) * 4
                for h in range(HPC):
                    def emit_scores(i):
                        sc = psA.tile([128, 1024], F32, tag="big")
                        for k2 in range(2):
                            sj = 2 * i + k2
                            blk, sb_ = sj // 4, sj % 4
                            nc.tensor.matmul(
                                sc[:, k2 * 512:(k2 + 1) * 512],
                                kT[blk][:, h, sb_ * 128:(sb_ + 1) * 128],
                                qT[ti][:, h, :], start=True, stop=True)
                        return sc

                    l_acc = accp.tile([128, TB], F32, tag="lacc")
                    o_ps = psB.tile([128, TB], F32, tag="o")
                    niter = ns // 2
                    sc_cur = emit_scores(0)
                    for i in range(niter):
                        sc_next = emit_scores(i + 1) if i + 1 < niter else None
                        probs = att.tile([128, 1024], BF, tag="probs")
                        nc.scalar.activation(probs[:], sc_cur[:], AF.Exp)
                        for k2 in range(2):
                            j = 2 * i + k2 - ti * 4
                            if j >= 0:
                                nc.vector.tensor_mul(
                                    probs[:, k2 * 512:(k2 + 1) * 512],
                                    probs[:, k2 * 512:(k2 + 1) * 512], mask_sb[:, j, :])
                        if i == 0:
                            nc.vector.tensor_add(l_acc[:], probs[:, 0:512], probs[:, 512:1024])
                        else:
                            nc.vector.tensor_add(l_acc[:], l_acc[:], probs[:, 0:512])
                            nc.vector.tensor_add(l_acc[:], l_acc[:], probs[:, 512:1024])
                        for k2 in range(2):
                            sj = 2 * i + k2
                            blk, sb_ = sj // 4, sj % 4
                            nc.tensor.matmul(
                                o_ps[:], vB[blk][:, sb_, h * 128:(h + 1) * 128],
                                probs[:, k2 * 512:(k2 + 1) * 512],
                                start=(sj == 0), stop=(sj == ns - 1))
                        sc_cur = sc_next
                    lcol = psC.tile([128, 4], F32, tag="tp")
                    for c in range(4):
                        nc.tensor.matmul(lcol[:, c:c + 1], l_acc[:, c * 128:(c + 1) * 128],
                                         ones[:], start=(c == 0), stop=(c == 3))
                    nc.vector.reciprocal(linv[:, h, :], lcol[:])
                    nc.any.tensor_copy(oB[:, h, :], o_ps[:])

                for sub in range(4):
                    out_sb = prj.tile([128, D], F32, tag="outsb")
                    for dn in range(D // 512):
                        pr0 = psD.tile([128, 512], F32, tag="pr")
                        nc.tensor.matmul(pr0[:], oB[:, 0, sub * 128:(sub + 1) * 128],
                                         wp_sb[:, 0, dn * 512:(dn + 1) * 512], start=True, stop=True)
                        tmp = prj.tile([128, 512], F32, tag="tmp")
                        nc.any.tensor_scalar_mul(tmp[:], pr0[:], linv[:, 0, sub:sub + 1])
                        pr1 = psD.tile([128, 512], F32, tag="pr")
                        nc.tensor.matmul(pr1[:], oB[:, 1, sub * 128:(sub + 1) * 128],
                                         wp_sb[:, 1, dn * 512:(dn + 1) * 512], start=True, stop=True)
                        nc.vector.scalar_tensor_tensor(
                            out_sb[:, dn * 512:(dn + 1) * 512], pr1[:], linv[:, 1, sub:sub + 1],
                            tmp[:], op0=ALU.mult, op1=ALU.add)
                    nc.sync.dma_start(out_d[t0 + sub * 128: t0 + (sub + 1) * 128, :], out_sb[:])
    return nc


def _host_prep(x, ve, lambdas, qkv_w, proj_w, T):
    x = np.asarray(x, np.float32).reshape(T, D)
    xt = np.ascontiguousarray(x.T).astype(bf16)  # [D, T] pre-transposed
    ve = np.asarray(ve, np.float32).reshape(T, NH * DH)
    lam = np.asarray(lambdas, np.float32)
    qkv_w = np.asarray(qkv_w, np.float32)
    proj_w = np.asarray(proj_w, np.float32)

    quarter = DH // 4
    ang = (1.0 / 1024.0) ** np.linspace(0.0, 1.0, quarter, dtype=np.float32)
    theta = np.arange(T, dtype=np.float32)[:, None] * ang[None, :]
    cos_t = np.cos(theta).astype(np.float32)
    sin_t = np.sin(theta).astype(np.float32)

    s_l = np.arange(128)[:, None]
    t_l = np.arange(TB)[None, :]
    mask = np.stack([(t_l >= s_l + 128 * j) for j in range(4)], axis=1).astype(bf16)

    in_maps = []
    for c in range(NCORES):
        sl = slice(c * DLOC, (c + 1) * DLOC)
        wqkv = np.concatenate(
            [qkv_w[0, sl].T, qkv_w[1, sl].T, lam[0] * qkv_w[2, sl].T], axis=1)
        in_maps.append({
            "xt": xt,
            "wqkv": np.ascontiguousarray(wqkv).astype(bf16),
            "wproj": np.ascontiguousarray(proj_w[:, sl].T).astype(bf16),
            "ve": np.ascontiguousarray(lam[1] * ve[:, sl]).astype(bf16),
            "cos": cos_t, "sin": sin_t, "mask": mask,
        })
    return in_maps


def kernel(x, ve, lambdas, qkv_w, proj_w):
    B, T, _ = x.shape
    in_maps = _host_prep(x, ve, lambdas, qkv_w, proj_w, T)
    if T not in _BUILD_CACHE:
        nc = _build(T)
        nc.compile()
        _BUILD_CACHE[T] = nc
    nc = _BUILD_CACHE[T]

    from concourse.bass_utils import run_bass_kernel_spmd
    res = run_bass_kernel_spmd(nc, in_maps, core_ids=list(range(NCORES)))
    out = np.zeros((T, D), np.float32)
    for c in range(NCORES):
        out += res.results[c]["out"]
    return out.reshape(B, T, D)
